# revision 1
# baseline (speedup 1.0000x reference)
"""Cross-attention (B=4, N=2048, C=768, H=12, HD=64) on 8 TRN2 NeuronCores.

Sharding: core = (batch, head_group) with 4 batches x 2 groups of 6 heads
(data parallel over batch, tensor parallel over heads).  Each core computes
its group's Q/K/V projections, per-head-dim LayerNorm, attention, and a
partial output projection; the host sums the two group partials per batch
and adds the bias.

Device-side layout notes:
 - Activations are fed pre-transposed (c on partitions) so every matmul
   contracts over the partition dim without any on-device transposes.
 - q~ / k~ live as [384, 2048] (head-dim on partitions), so attention
   scores are computed transposed: S^T[k_tok, q_tok].  Softmax exp needs
   no row-max (LN bounds |S| < ~10), masked q rows are folded into the
   LN scale (rs *= mask) making their score columns exactly 0 -> uniform
   softmax, matching the reference's -1e9 row-fill semantics.
 - The softmax denominator comes free from a ones-column appended to v
   (PV matmul lhsT is [128, 65]; row 64 accumulates sum_j E[j, i]).
 - All matmuls run as float32r (full PE rate at moving dim >= 256,
   ~1e-4 relative error).  Producers write through f32r-bitcast APs to
   satisfy the compiler's "rounded to FP32r" rule.
"""

import numpy as np

import concourse.bass as bass
import concourse.mybir as mybir
from concourse import tile
from concourse import bass_utils
from concourse.tile_scheduler import N_PROCS
from concourse.vector_clock import ScopedClock, VectorClock

F32 = mybir.dt.float32
F32R = mybir.dt.float32r
AF = mybir.ActivationFunctionType
OP = mybir.AluOpType

B, N, C, H, HD = 4, 2048, 768, 12, 64
G = 2                 # head groups (tensor parallel)
HPG = H // G          # 6 heads per group
CL = HPG * HD         # 384 local channels
P = 128
CH = 512              # token chunk
NCH = N // CH         # 4
NT = CL // P          # 3 output tiles per group
CT = C // P           # 6 contraction tiles
TT = N // P           # 16 token tiles
KT_GRP = 2            # k-tiles per exp group ([128, 1024] S^T psum)
EPS = 1e-5
SCALE = HD ** -0.5
NCORES = 8

_nop_ctr = [0]


class _FixedTileContext(tile.TileContext):
    """Workaround for a walrus build that allows at most ONE sync-wait per
    instruction: split multi-wait instructions into single-wait NoOps on the
    same engine, and emit the kernel-tail drain's waits as a nop chain."""

    def _split_multiwait(self, insts):
        out = []
        for inst in insts:
            si = getattr(inst, "sync_info", None)
            waits = list(si.on_wait) if si is not None and si.on_wait else []
            if len(waits) > 1:
                eng = inst.engine
                for w in waits[:-1]:
                    _nop_ctr[0] += 1
                    nop = mybir.InstNoOp(
                        name=f"I-waitsplit-{_nop_ctr[0]}", ins=[], outs=[]
                    )
                    nop.engine = eng
                    nop.sync_info = mybir.SyncInfo(on_wait=[w], on_update=[])
                    self.nc.register_instruction(nop)
                    out.append(nop)
                inst.sync_info = mybir.SyncInfo(
                    on_wait=[waits[-1]], on_update=list(si.on_update)
                )
            out.append(inst)
        return out

    def _lower_ordered_insts(self, ordered):
        ordered = {bb: self._split_multiwait(ins) for bb, ins in ordered.items()}
        super()._lower_ordered_insts(ordered)

    def _drain_and_barrier(self, tick_clock, wait_clock):
        gc = tick_clock.global_clock
        vals = [gc[p] for p in range(N_PROCS)]
        for p in [q for q, v in enumerate(vals) if v > 0]:
            partial = VectorClock(
                [vals[q] if q == p else 0 for q in range(N_PROCS)]
            )
            nop = self.nc.sync.nop(nofuse=True, hint="tail_drain_wait")
            wait_clock.add_sem_waits(nop.ins, ScopedClock({None: partial}))
        self.nc.sync.drain()
        self.nc.all_engine_barrier()
        assert self.sems is not None
        popped = self.nc._tile_sem_poison_stack.pop()
        assert popped is self._sem_poison
        self.nc.clear_and_free_semaphores(list(self.sems.allocated().values()))
        self.nc.all_engine_barrier()


def _mm(nc, out, lhsT, rhs, start, stop):
    nc.tensor.matmul(
        out, lhsT, rhs, start=start, stop=stop, skip_group_check=True
    )


def _body(tc, aps):
    nc = tc.nc
    qxT, kvxT, wq, wk, wv, wp, msk, colsel, bcast, ones1, vones, outT = aps

    cpool = tc.alloc_tile_pool(name="consts", bufs=1)
    bpool = tc.alloc_tile_pool(name="big", bufs=1)

    colsel_sb = cpool.tile([P, NT, HPG], F32R, name="colsel", tag="colsel")
    nc.sync.dma_start(colsel_sb[:], colsel[:])
    bcast_sb = cpool.tile([HPG, NT, P], F32R, name="bcast", tag="bcast")
    nc.sync.dma_start(bcast_sb[:], bcast[:])
    ones4_sb = cpool.tile([65, HD], F32R, name="ones4", tag="ones4")
    nc.sync.dma_start(ones4_sb[:], ones1[:])
    msk_sb = cpool.tile([HPG, N], F32, name="msk", tag="msk")
    nc.sync.dma_start(msk_sb[:], msk[:])
    eps_sb = cpool.tile([HPG, 1], F32, name="eps", tag="eps")
    nc.vector.memset(eps_sb[:], EPS)

    q_sb = [bpool.tile([P, N], F32, name=f"q{t}", tag=f"q{t}") for t in range(NT)]
    k_sb = [bpool.tile([P, N], F32, name=f"k{t}", tag=f"k{t}") for t in range(NT)]
    v_sb = bpool.tile([P, TT, HPG, HD + 1], F32, name="v", tag="v")
    den_all = bpool.tile([65, HPG * CH], F32, name="den", tag="den")

    # ---------------- phase 1: projections + layernorm ----------------
    ps_t = tc.alloc_tile_pool(name="ps1", bufs=8, space="PSUM")
    w_pool = tc.alloc_tile_pool(name="wts", bufs=1)
    xq_pool = tc.alloc_tile_pool(name="xq", bufs=3)
    xkv_pool = tc.alloc_tile_pool(name="xkv", bufs=7)
    sq_pool = tc.alloc_tile_pool(name="sq", bufs=3)
    st_pool = tc.alloc_tile_pool(name="st", bufs=2)
    if True:
        wq_sb = w_pool.tile([P, CT, CL], F32R, name="wq", tag="wq")
        nc.sync.dma_start(wq_sb[:], wq.rearrange("(ct p) m -> p ct m", p=P))
        wk_sb = w_pool.tile([P, CT, CL], F32R, name="wk", tag="wk")
        wv_sb = w_pool.tile([P, CT, CL], F32R, name="wv", tag="wv")

        def ln_chunk(xT, w_sb, dst, masked, c):
            if True:
                cs = slice(c * CH, (c + 1) * CH)
                pp = [ps_t.tile([P, CH], F32, name="pt", tag="pt") for _ in range(NT)]
                xts = []
                pool = xq_pool if masked else xkv_pool
                xtag = "xq" if masked else "xkv"
                for ct in range(CT):
                    xt = pool.tile([P, CH], F32R, name=xtag, tag=xtag)
                    nc.sync.dma_start(xt[:], xT[ct * P:(ct + 1) * P, cs])
                    xts.append(xt)
                    for t in range(NT):
                        _mm(nc, pp[t][:], w_sb[:, ct, t * P:(t + 1) * P],
                            xt[:], ct == 0, ct == CT - 1)
                sqs = []
                for t in range(NT):
                    nc.vector.tensor_copy(dst[t][:, cs].bitcast(F32R), pp[t][:])
                    sq_t = sq_pool.tile([P, CH], F32, name="sq", tag="sq")
                    nc.scalar.activation(sq_t[:].bitcast(F32R), pp[t][:], AF.Square)
                    sqs.append(sq_t)
                mu_ps = ps_t.tile([HPG, CH], F32, name="pt", tag="pt")
                for t in range(NT):
                    _mm(nc, mu_ps[:], colsel_sb[:, t, :],
                        dst[t][:, cs].bitcast(F32R), t == 0, t == NT - 1)
                ms_ps = ps_t.tile([HPG, CH], F32, name="pt", tag="pt")
                for t in range(NT):
                    _mm(nc, ms_ps[:], colsel_sb[:, t, :],
                        sqs[t][:].bitcast(F32R), t == 0, t == NT - 1)
                st = st_pool.tile([HPG, 4 * CH], F32, name="st", tag="st")
                work = st[:, 0:CH]
                rs = st[:, CH:2 * CH]
                murs = st[:, 2 * CH:3 * CH]
                mu_sb = st[:, 3 * CH:4 * CH]
                nc.vector.tensor_copy(mu_sb.bitcast(F32R), mu_ps[:])
                # var = E[x^2] - mu^2
                nc.vector.scalar_tensor_tensor(
                    work.bitcast(F32R), mu_sb, 1.0, mu_sb, OP.mult, OP.mult)
                nc.vector.tensor_tensor(
                    work.bitcast(F32R), ms_ps[:], work, OP.subtract)
                # rs = (var + eps)^-0.5 = exp(-0.5 * ln(var + eps))
                nc.scalar.activation(murs.bitcast(F32R), work, AF.Ln,
                                     bias=eps_sb[:])
                if masked:
                    nc.scalar.activation(rs.bitcast(F32R), murs, AF.Exp,
                                         scale=-0.5)
                    # fold attn scale + query mask into rs
                    nc.vector.tensor_tensor(
                        rs.bitcast(F32R), rs, msk_sb[:, cs], OP.mult)
                else:
                    nc.scalar.activation(rs.bitcast(F32R), murs, AF.Exp,
                                         scale=-0.5)
                # murs = -mu * rs
                nc.vector.scalar_tensor_tensor(
                    murs.bitcast(F32R), mu_sb, -1.0, rs, OP.mult, OP.mult)
                for t in range(NT):
                    rrep = ps_t.tile([P, CH], F32, name="pt", tag="pt")
                    _mm(nc, rrep[:], bcast_sb[:, t, :], rs.bitcast(F32R),
                        True, True)
                    mrep = ps_t.tile([P, CH], F32, name="pt", tag="pt")
                    _mm(nc, mrep[:], bcast_sb[:, t, :], murs.bitcast(F32R),
                        True, True)
                    nc.vector.tensor_tensor(
                        dst[t][:, cs].bitcast(F32R), dst[t][:, cs], rrep[:],
                        OP.mult)
                    nc.vector.tensor_tensor(
                        dst[t][:, cs].bitcast(F32R), dst[t][:, cs], mrep[:],
                        OP.add)
                if not masked:
                    # v projection reuses this chunk's kv x-tiles
                    for tl in range(CH // P):
                        tt = c * (CH // P) + tl
                        vp = ps_t.tile([P, CL], F32, name="pt", tag="pt")
                        for ct in range(CT):
                            _mm(nc, vp[:], xts[ct][:, tl * P:(tl + 1) * P],
                                wv_sb[:, ct, :], ct == 0, ct == CT - 1)
                        nc.vector.tensor_copy(
                            v_sb[:, tt, :, 0:HD].bitcast(F32R),
                            vp[:].rearrange("p (h d) -> p h d", h=HPG))

        for c in range(NCH):
            ln_chunk(qxT, wq_sb, q_sb, True, c)
            if c == 0:
                nc.sync.dma_start(
                    wk_sb[:], wk.rearrange("(ct p) m -> p ct m", p=P))
                nc.sync.dma_start(
                    wv_sb[:], wv.rearrange("(ct p) m -> p ct m", p=P))
                nc.sync.dma_start(v_sb[:, :, :, HD].bitcast(F32R), vones[:])
            ln_chunk(kvxT, wk_sb, k_sb, False, c)

    for pool in (st_pool, sq_pool, xkv_pool, xq_pool, w_pool, ps_t):
        pool.release()

    # ---------------- phase 2: attention + output projection ----------
    ps_s = tc.alloc_tile_pool(name="ps_s", bufs=2, space="PSUM")
    ps_o = tc.alloc_tile_pool(name="ps_o", bufs=2, space="PSUM")
    ps_t = tc.alloc_tile_pool(name="ps2", bufs=2, space="PSUM")
    wp_pool = tc.alloc_tile_pool(name="wp", bufs=1)
    e_pool = tc.alloc_tile_pool(name="e", bufs=4)
    o_pool = tc.alloc_tile_pool(name="o", bufs=2)
    rcp_pool = tc.alloc_tile_pool(name="rcp", bufs=2)
    out_pool = tc.alloc_tile_pool(name="ot", bufs=3)
    if True:
        wp_sb = wp_pool.tile([P, NT, C], F32R, name="wp", tag="wp")
        nc.sync.dma_start(wp_sb[:], wp.rearrange("(t p) m -> p t m", p=P))
        for qc in range(NCH):
            qs = slice(qc * CH, (qc + 1) * CH)
            o_t = [o_pool.tile([P, CH], F32, name=f"o{t}", tag=f"o{t}") for t in range(NT)]
            for h in range(HPG):
                t, off = h // 2, (h % 2) * HD
                po = ps_o.tile([HD + 1, CH], F32, name="po", tag="po")
                for kg in range(TT // KT_GRP):
                    sp = ps_s.tile([P, KT_GRP * CH], F32, name="sp", tag="sp")
                    for j in range(KT_GRP):
                        kt = kg * KT_GRP + j
                        _mm(nc, sp[:, j * CH:(j + 1) * CH],
                            k_sb[t][off:off + HD, kt * P:(kt + 1) * P].bitcast(F32R),
                            q_sb[t][off:off + HD, qs].bitcast(F32R),
                            True, True)
                    e = e_pool.tile([P, KT_GRP * CH], F32, name="e", tag="e")
                    nc.scalar.activation(e[:].bitcast(F32R), sp[:], AF.Exp)
                    for j in range(KT_GRP):
                        kt = kg * KT_GRP + j
                        _mm(nc, po[:], v_sb[:, kt, h, :].bitcast(F32R),
                            e[:, j * CH:(j + 1) * CH].bitcast(F32R),
                            kt == 0, kt == TT - 1)
                # stash denominator (po row 64) and raw O rows; the
                # normalize happens after the qc's batched reciprocal.
                db = 32 * (qc % 3)
                nc.vector.tensor_copy(
                    den_all[db:db + 1, h * CH:(h + 1) * CH].bitcast(F32R),
                    po[HD:HD + 1, :])
                nc.vector.tensor_copy(
                    o_t[t][off:off + HD, :].bitcast(F32R), po[0:HD, :])
            # batched reciprocal for all 6 heads of this qc: repack the
            # [1, 6*CH] denominator row into [32, 96] (DVE reciprocal cost
            # scales with free size only), invert, and scatter back.
            db = 32 * (qc % 3)
            dpk = rcp_pool.tile([32, HPG * CH // 32], F32, name="dpk", tag="dpk")
            nc.sync.dma_start(dpk[:], den_all[db:db + 1, :])
            rpk = rcp_pool.tile([32, HPG * CH // 32], F32, name="rpk", tag="rpk")
            nc.vector.reciprocal(rpk[:], dpk[:])
            nc.sync.dma_start(
                den_all[db:db + 1, :].bitcast(F32R), rpk[:].bitcast(F32R))
            for h in range(HPG):
                t, off = h // 2, (h % 2) * HD
                rrep = ps_t.tile([HD, CH], F32, name="pt", tag="pt")
                _mm(nc, rrep[:], ones4_sb[db:db + 1, :],
                    den_all[db:db + 1, h * CH:(h + 1) * CH].bitcast(F32R),
                    True, True)
                nc.vector.tensor_tensor(
                    o_t[t][off:off + HD, :].bitcast(F32R),
                    o_t[t][off:off + HD, :], rrep[:], OP.mult)
            for m in range(C // P):
                pp = ps_t.tile([P, CH], F32, name="pt", tag="pt")
                for t in range(NT):
                    _mm(nc, pp[:], wp_sb[:, t, m * P:(m + 1) * P],
                        o_t[t][:].bitcast(F32R), t == 0, t == NT - 1)
                ot = out_pool.tile([P, CH], F32, name="ot", tag="ot")
                nc.vector.tensor_copy(ot[:], pp[:])
                nc.sync.dma_start(outT[m * P:(m + 1) * P, qs], ot[:])

    for pool in (out_pool, rcp_pool, o_pool, e_pool, wp_pool,
                 ps_t, ps_o, ps_s, bpool, cpool):
        pool.release()


def build_bass():
    nc = bass.Bass(trn_type="TRN2", debug=False, num_devices=NCORES)
    qxT = nc.dram_tensor("qxT", [C, N], F32R, kind="ExternalInput").ap()
    kvxT = nc.dram_tensor("kvxT", [C, N], F32R, kind="ExternalInput").ap()
    wq = nc.dram_tensor("wq", [C, CL], F32R, kind="ExternalInput").ap()
    wk = nc.dram_tensor("wk", [C, CL], F32R, kind="ExternalInput").ap()
    wv = nc.dram_tensor("wv", [C, CL], F32R, kind="ExternalInput").ap()
    wp = nc.dram_tensor("wp", [CL, C], F32R, kind="ExternalInput").ap()
    msk = nc.dram_tensor("msk", [HPG, N], F32, kind="ExternalInput").ap()
    colsel = nc.dram_tensor("colsel", [P, NT, HPG], F32R,
                            kind="ExternalInput").ap()
    bcast = nc.dram_tensor("bcast", [HPG, NT, P], F32R,
                           kind="ExternalInput").ap()
    ones1 = nc.dram_tensor("ones1", [65, HD], F32R, kind="ExternalInput").ap()
    vones = nc.dram_tensor("vones", [P, TT, HPG], F32R,
                           kind="ExternalInput").ap()
    outT = nc.dram_tensor("outT", [C, N], F32, kind="ExternalOutput").ap()
    aps = (qxT, kvxT, wq, wk, wv, wp, msk, colsel, bcast, ones1, vones, outT)
    with _FixedTileContext(nc) as tc:
        _body(tc, aps)
    return nc


def make_in_maps(q_x, kv_x, attn_mask, Wq, Wkv, Wp):
    colsel = np.zeros((P, NT, HPG), np.float32)
    bcast = np.zeros((HPG, NT, P), np.float32)
    for t in range(NT):
        for pp in range(P):
            colsel[pp, t, 2 * t + pp // HD] = 1.0 / HD
            bcast[2 * t + pp // HD, t, pp] = 1.0
    ones1 = np.zeros((65, HD), np.float32)
    ones1[[0, 32, 64], :] = 1.0

    in_maps = []
    for core in range(NCORES):
        b, g = core // G, core % G
        sl = slice(g * CL, (g + 1) * CL)
        in_maps.append({
            "qxT": np.ascontiguousarray(q_x[b].T),
            "kvxT": np.ascontiguousarray(kv_x[b].T),
            "wq": np.ascontiguousarray(Wq[sl].T),
            "wk": np.ascontiguousarray(Wkv[sl].T),
            "wv": np.ascontiguousarray(Wkv[C + g * CL:C + (g + 1) * CL].T),
            "wp": np.ascontiguousarray(Wp[:, sl].T),
            "msk": np.broadcast_to(
                attn_mask[b].astype(np.float32) * SCALE, (HPG, N)).copy(),
            "colsel": colsel,
            "bcast": bcast,
            "ones1": ones1,
            "vones": np.ones((P, TT, HPG), np.float32),
        })
    return in_maps


_NC_CACHE = []


def get_nc():
    if not _NC_CACHE:
        _NC_CACHE.append(build_bass())
    return _NC_CACHE[0]


def kernel(q_x, kv_x, attn_mask, Wq, Wkv, qn_w, qn_b, kn_w, kn_b, Wp, bp,
           _profile=None):
    q_x = np.asarray(q_x, np.float32)
    kv_x = np.asarray(kv_x, np.float32)
    attn_mask = np.asarray(attn_mask)
    Wq = np.asarray(Wq, np.float32)
    Wkv = np.asarray(Wkv, np.float32)
    Wp = np.asarray(Wp, np.float32)
    bp = np.asarray(bp, np.float32)
    if not (np.all(np.asarray(qn_w) == 1) and np.all(np.asarray(qn_b) == 0)
            and np.all(np.asarray(kn_w) == 1) and np.all(np.asarray(kn_b) == 0)):
        raise NotImplementedError("kernel specialized to identity q/k norms")

    nc = get_nc()
    in_maps = make_in_maps(q_x, kv_x, attn_mask, Wq, Wkv, Wp)
    res = bass_utils.run_bass_kernel_spmd(
        nc, in_maps, core_ids=list(range(NCORES)))
    if _profile is not None:
        _profile.append(res)
    out = np.empty((B, N, C), np.float32)
    for b in range(B):
        acc = res.results[G * b]["outT"] + res.results[G * b + 1]["outT"]
        out[b] = acc.T + bp
    return out



# revision 3
# speedup vs baseline: 1.7335x; 1.7335x over previous
"""Cross-attention (B=4, N=2048, C=768, H=12, HD=64) on 8 TRN2 NeuronCores.

Sharding: core = (batch, head_group) with 4 batches x 2 groups of 6 heads.
Each core computes its group's Q/K/V projections, per-head-dim LayerNorm,
attention, and a partial output projection; the host sums the two group
partials per batch and adds the bias.

Key optimizations over the f32r baseline:
 - All matmul operands are bf16 (PSUM accumulation stays f32).  The PE
   processes one moving row per cycle either way, but bf16 halves SBUF
   footprint, weight-load time, and DMA traffic.
 - Query-token compaction: the reference masks ~50% of QUERY rows, and a
   masked row's output is exactly the uniform average of v (its score row
   is all zeros after the mask is folded into the LN scale).  The host
   gathers unmasked q tokens, the kernel runs attention on only NQ ~ 1280
   columns, and any PAD column (rs=0) computes precisely the uniform-
   attention output -- so the host reads column cnt_b as the shared
   output for all masked tokens of batch b, then scatters.
 - Attention scores are computed transposed (S^T[k_tok, q_tok]); softmax
   exp needs no row-max (LN bounds |S|); the denominator comes free from
   a ones-column appended to v.
"""

import numpy as np
import ml_dtypes

import concourse.bass as bass
import concourse.mybir as mybir
from concourse import tile
from concourse import bass_utils
from concourse.tile_scheduler import N_PROCS
from concourse.vector_clock import ScopedClock, VectorClock

F32 = mybir.dt.float32
BF16 = mybir.dt.bfloat16
AF = mybir.ActivationFunctionType
OP = mybir.AluOpType
NPBF16 = ml_dtypes.bfloat16

B, N, C, H, HD = 4, 2048, 768, 12, 64
G = 2                 # head groups (tensor parallel)
HPG = H // G          # 6 heads per group
CL = HPG * HD         # 384 local channels
P = 128
NT = CL // P          # 3 output tiles per group
CT = C // P           # 6 contraction tiles
TT = N // P           # 16 k-token tiles
EPS = 1e-5
SCALE = HD ** -0.5
NCORES = 8

_nop_ctr = [0]


class _FixedTileContext(tile.TileContext):
    """Workaround for a walrus build that allows at most ONE sync-wait per
    instruction: split multi-wait instructions into single-wait NoOps on the
    same engine, and emit the kernel-tail drain's waits as a nop chain."""

    def _split_multiwait(self, insts):
        out = []
        for inst in insts:
            si = getattr(inst, "sync_info", None)
            waits = list(si.on_wait) if si is not None and si.on_wait else []
            if len(waits) > 1:
                eng = inst.engine
                for w in waits[:-1]:
                    _nop_ctr[0] += 1
                    nop = mybir.InstNoOp(
                        name=f"I-waitsplit-{_nop_ctr[0]}", ins=[], outs=[]
                    )
                    nop.engine = eng
                    nop.sync_info = mybir.SyncInfo(on_wait=[w], on_update=[])
                    self.nc.register_instruction(nop)
                    out.append(nop)
                inst.sync_info = mybir.SyncInfo(
                    on_wait=[waits[-1]], on_update=list(si.on_update)
                )
            out.append(inst)
        return out

    def _lower_ordered_insts(self, ordered):
        ordered = {bb: self._split_multiwait(ins) for bb, ins in ordered.items()}
        super()._lower_ordered_insts(ordered)

    def _drain_and_barrier(self, tick_clock, wait_clock):
        gc = tick_clock.global_clock
        vals = [gc[p] for p in range(N_PROCS)]
        for p in [q for q, v in enumerate(vals) if v > 0]:
            partial = VectorClock(
                [vals[q] if q == p else 0 for q in range(N_PROCS)]
            )
            nop = self.nc.sync.nop(nofuse=True, hint="tail_drain_wait")
            wait_clock.add_sem_waits(nop.ins, ScopedClock({None: partial}))
        self.nc.sync.drain()
        self.nc.all_engine_barrier()
        assert self.sems is not None
        popped = self.nc._tile_sem_poison_stack.pop()
        assert popped is self._sem_poison
        self.nc.clear_and_free_semaphores(list(self.sems.allocated().values()))
        self.nc.all_engine_barrier()


def _mm(nc, out, lhsT, rhs, start, stop):
    nc.tensor.matmul(
        out, lhsT, rhs, start=start, stop=stop, skip_group_check=True
    )


def _chunks(total):
    """Split token range into chunks of 512 with a trailing 256 if needed."""
    out = []
    off = 0
    while off < total:
        w = 512 if total - off >= 512 else total - off
        out.append((off, w))
        off += w
    return out


def _body(tc, aps, nq):
    nc = tc.nc
    qxT, kvxT, wq, wk, wv, wp, msk, colsel, bcast, ones1, vones, outT = aps

    cpool = tc.alloc_tile_pool(name="consts", bufs=1)
    bpool = tc.alloc_tile_pool(name="big", bufs=1)

    colsel_sb = cpool.tile([P, NT, HPG], BF16, name="colsel", tag="colsel")
    nc.sync.dma_start(colsel_sb[:], colsel[:])
    bcast_sb = cpool.tile([HPG, NT, P], BF16, name="bcast", tag="bcast")
    nc.sync.dma_start(bcast_sb[:], bcast[:])
    ones4_sb = cpool.tile([65, HD], BF16, name="ones4", tag="ones4")
    nc.sync.dma_start(ones4_sb[:], ones1[:])
    msk_sb = cpool.tile([HPG, nq], F32, name="msk", tag="msk")
    nc.sync.dma_start(msk_sb[:], msk[:])
    eps_sb = cpool.tile([HPG, 1], F32, name="eps", tag="eps")
    nc.vector.memset(eps_sb[:], EPS)

    q_sb = [bpool.tile([P, nq], BF16, name=f"q{t}", tag=f"q{t}") for t in range(NT)]
    k_sb = [bpool.tile([P, N], BF16, name=f"k{t}", tag=f"k{t}") for t in range(NT)]
    v_sb = bpool.tile([P, TT, HPG, HD + 1], BF16, name="v", tag="v")
    den_all = bpool.tile([65, HPG * 512], F32, name="den", tag="den")
    rcp_all = bpool.tile([65, HPG * 512], BF16, name="rcp", tag="rcp")

    q_chunks = _chunks(nq)
    k_chunks = _chunks(N)

    # ---------------- phase 1: projections + layernorm ----------------
    ps_t = tc.alloc_tile_pool(name="ps1", bufs=8, space="PSUM")
    w_pool = tc.alloc_tile_pool(name="wts", bufs=1)
    xq_pool = tc.alloc_tile_pool(name="xq", bufs=3)
    xkv_pool = tc.alloc_tile_pool(name="xkv", bufs=7)
    sq_pool = tc.alloc_tile_pool(name="sq", bufs=3)
    st_pool = tc.alloc_tile_pool(name="st", bufs=2)
    rs_pool = tc.alloc_tile_pool(name="rs", bufs=2)
    if True:
        wq_sb = w_pool.tile([P, CT, CL], BF16, name="wq", tag="wq")
        nc.sync.dma_start(wq_sb[:], wq.rearrange("(ct p) m -> p ct m", p=P))
        wk_sb = w_pool.tile([P, CT, CL], BF16, name="wk", tag="wk")
        wv_sb = w_pool.tile([P, CT, CL], BF16, name="wv", tag="wv")

        def ln_chunk(xT, w_sb, dst, masked, co, cw):
            cs = slice(co, co + cw)
            pp = [ps_t.tile([P, cw], F32, name="pt", tag="pt") for _ in range(NT)]
            xts = []
            pool = xq_pool if masked else xkv_pool
            xtag = "xq" if masked else "xkv"
            for ct in range(CT):
                xt = pool.tile([P, cw], BF16, name=xtag, tag=xtag)
                nc.sync.dma_start(xt[:], xT[ct * P:(ct + 1) * P, cs])
                xts.append(xt)
                for t in range(NT):
                    _mm(nc, pp[t][:], w_sb[:, ct, t * P:(t + 1) * P],
                        xt[:], ct == 0, ct == CT - 1)
            sqs = []
            for t in range(NT):
                nc.vector.tensor_copy(dst[t][:, cs], pp[t][:])
                sq_t = sq_pool.tile([P, cw], BF16, name="sq", tag="sq")
                nc.scalar.activation(sq_t[:], pp[t][:], AF.Square)
                sqs.append(sq_t)
            mu_ps = ps_t.tile([HPG, cw], F32, name="pt", tag="pt")
            for t in range(NT):
                _mm(nc, mu_ps[:], colsel_sb[:, t, :], dst[t][:, cs],
                    t == 0, t == NT - 1)
            ms_ps = ps_t.tile([HPG, cw], F32, name="pt", tag="pt")
            for t in range(NT):
                _mm(nc, ms_ps[:], colsel_sb[:, t, :], sqs[t][:],
                    t == 0, t == NT - 1)
            st = st_pool.tile([HPG, 2 * cw], F32, name="st", tag="st")
            work = st[:, 0:cw]
            mu_sb = st[:, cw:2 * cw]
            rsm = rs_pool.tile([HPG, 2, cw], BF16, name="rsm", tag="rsm")
            rs = rsm[:, 0, :]
            murs = rsm[:, 1, :]
            nc.vector.tensor_copy(mu_sb, mu_ps[:])
            # var = E[x^2] - mu^2
            nc.vector.scalar_tensor_tensor(
                work, mu_sb, 1.0, mu_sb, OP.mult, OP.mult)
            nc.vector.tensor_tensor(work, ms_ps[:], work, OP.subtract)
            # rs = (var + eps)^-0.5 = exp(-0.5 * ln(var + eps))
            nc.scalar.activation(work, work, AF.Ln, bias=eps_sb[:])
            if masked:
                nc.scalar.activation(work, work, AF.Exp, scale=-0.5)
                # fold attn scale + query mask into rs
                nc.vector.tensor_tensor(rs, work, msk_sb[:, cs], OP.mult)
            else:
                nc.scalar.activation(rs, work, AF.Exp, scale=-0.5)
            # murs = -mu * rs
            nc.vector.scalar_tensor_tensor(
                murs, mu_sb, -1.0, rs, OP.mult, OP.mult)
            for t in range(NT):
                rrep = ps_t.tile([P, cw], F32, name="pt", tag="pt")
                _mm(nc, rrep[:], bcast_sb[:, t, :], rs, True, True)
                mrep = ps_t.tile([P, cw], F32, name="pt", tag="pt")
                _mm(nc, mrep[:], bcast_sb[:, t, :], murs, True, True)
                nc.vector.tensor_tensor(
                    dst[t][:, cs], dst[t][:, cs], rrep[:], OP.mult)
                nc.vector.tensor_tensor(
                    dst[t][:, cs], dst[t][:, cs], mrep[:], OP.add)
            if not masked:
                # v projection reuses this chunk's kv x-tiles
                for tl in range(cw // P):
                    tt = co // P + tl
                    vp = ps_t.tile([P, CL], F32, name="pt", tag="pt")
                    for ct in range(CT):
                        _mm(nc, vp[:], xts[ct][:, tl * P:(tl + 1) * P],
                            wv_sb[:, ct, :], ct == 0, ct == CT - 1)
                    nc.vector.tensor_copy(
                        v_sb[:, tt, :, 0:HD],
                        vp[:].rearrange("p (h d) -> p h d", h=HPG))

        for c in range(len(k_chunks)):
            if c < len(q_chunks):
                ln_chunk(qxT, wq_sb, q_sb, True, *q_chunks[c])
            if c == 0:
                nc.sync.dma_start(
                    wk_sb[:], wk.rearrange("(ct p) m -> p ct m", p=P))
                nc.sync.dma_start(
                    wv_sb[:], wv.rearrange("(ct p) m -> p ct m", p=P))
                nc.sync.dma_start(v_sb[:, :, :, HD], vones[:])
            ln_chunk(kvxT, wk_sb, k_sb, False, *k_chunks[c])

    for pool in (rs_pool, st_pool, sq_pool, xkv_pool, xq_pool, w_pool, ps_t):
        pool.release()

    # ---------------- phase 2: attention + output projection ----------
    ps_s = tc.alloc_tile_pool(name="ps_s", bufs=2, space="PSUM")
    ps_o = tc.alloc_tile_pool(name="ps_o", bufs=2, space="PSUM")
    ps_t = tc.alloc_tile_pool(name="ps2", bufs=2, space="PSUM")
    wp_pool = tc.alloc_tile_pool(name="wp", bufs=1)
    e_pool = tc.alloc_tile_pool(name="e", bufs=4)
    o_pool = tc.alloc_tile_pool(name="o", bufs=2)
    rcp_pool = tc.alloc_tile_pool(name="rcp", bufs=2)
    out_pool = tc.alloc_tile_pool(name="ot", bufs=3)
    if True:
        wp_sb = wp_pool.tile([P, NT, C], BF16, name="wp", tag="wp")
        nc.sync.dma_start(wp_sb[:], wp.rearrange("(t p) m -> p t m", p=P))
        for qc, (co, cw) in enumerate(q_chunks):
            qs = slice(co, co + cw)
            kt_grp = 1024 // cw          # k-tiles per [128, 1024] exp group
            o_t = [o_pool.tile([P, cw], BF16, name=f"o{t}", tag=f"o{t}")
                   for t in range(NT)]
            for h in range(HPG):
                t, off = h // 2, (h % 2) * HD
                po = ps_o.tile([HD + 1, cw], F32, name="po", tag="po")
                for kg in range(TT // kt_grp):
                    sp = ps_s.tile([P, kt_grp * cw], F32, name="sp", tag="sp")
                    for j in range(kt_grp):
                        kt = kg * kt_grp + j
                        _mm(nc, sp[:, j * cw:(j + 1) * cw],
                            k_sb[t][off:off + HD, kt * P:(kt + 1) * P],
                            q_sb[t][off:off + HD, qs],
                            True, True)
                    e = e_pool.tile([P, kt_grp * cw], BF16, name="e", tag="e")
                    nc.scalar.activation(e[:], sp[:], AF.Exp)
                    for j in range(kt_grp):
                        kt = kg * kt_grp + j
                        _mm(nc, po[:], v_sb[:, kt, h, :],
                            e[:, j * cw:(j + 1) * cw],
                            kt == 0, kt == TT - 1)
                # stash denominator (po row 64) and raw O rows; the
                # normalize happens after the qc's batched reciprocal.
                db = 32 * (qc % 3)
                nc.vector.tensor_copy(
                    den_all[db:db + 1, h * cw:(h + 1) * cw], po[HD:HD + 1, :])
                nc.vector.tensor_copy(o_t[t][off:off + HD, :], po[0:HD, :])
            # batched reciprocal for all 6 heads of this qc: repack the
            # [1, 6*cw] denominator row into [32, :] (DVE reciprocal cost
            # scales with free size only), invert, and scatter back.
            db = 32 * (qc % 3)
            dpk = rcp_pool.tile([32, HPG * cw // 32], F32, name="dpk", tag="dpk")
            nc.sync.dma_start(dpk[:], den_all[db:db + 1, 0:HPG * cw])
            rpk = rcp_pool.tile([32, HPG * cw // 32], BF16, name="rpk", tag="rpk")
            with nc.allow_low_precision(reason="bf16 softmax denom recip"):
                nc.vector.reciprocal(rpk[:], dpk[:])
            nc.sync.dma_start(rcp_all[db:db + 1, 0:HPG * cw], rpk[:])
            for h in range(HPG):
                t, off = h // 2, (h % 2) * HD
                rrep = ps_t.tile([HD, cw], F32, name="pt", tag="pt")
                _mm(nc, rrep[:], ones4_sb[db:db + 1, :],
                    rcp_all[db:db + 1, h * cw:(h + 1) * cw], True, True)
                nc.vector.tensor_tensor(
                    o_t[t][off:off + HD, :], o_t[t][off:off + HD, :],
                    rrep[:], OP.mult)
            for m in range(C // P):
                pp = ps_t.tile([P, cw], F32, name="pt", tag="pt")
                for t in range(NT):
                    _mm(nc, pp[:], wp_sb[:, t, m * P:(m + 1) * P],
                        o_t[t][:], t == 0, t == NT - 1)
                ot = out_pool.tile([P, cw], F32, name="ot", tag="ot")
                nc.vector.tensor_copy(ot[:], pp[:])
                nc.sync.dma_start(outT[m * P:(m + 1) * P, qs], ot[:])

    for pool in (out_pool, rcp_pool, o_pool, e_pool, wp_pool,
                 ps_t, ps_o, ps_s, bpool, cpool):
        pool.release()


def build_bass(nq):
    nc = bass.Bass(trn_type="TRN2", debug=False, num_devices=NCORES)
    qxT = nc.dram_tensor("qxT", [C, nq], BF16, kind="ExternalInput").ap()
    kvxT = nc.dram_tensor("kvxT", [C, N], BF16, kind="ExternalInput").ap()
    wq = nc.dram_tensor("wq", [C, CL], BF16, kind="ExternalInput").ap()
    wk = nc.dram_tensor("wk", [C, CL], BF16, kind="ExternalInput").ap()
    wv = nc.dram_tensor("wv", [C, CL], BF16, kind="ExternalInput").ap()
    wp = nc.dram_tensor("wp", [CL, C], BF16, kind="ExternalInput").ap()
    msk = nc.dram_tensor("msk", [HPG, nq], F32, kind="ExternalInput").ap()
    colsel = nc.dram_tensor("colsel", [P, NT, HPG], BF16,
                            kind="ExternalInput").ap()
    bcast = nc.dram_tensor("bcast", [HPG, NT, P], BF16,
                           kind="ExternalInput").ap()
    ones1 = nc.dram_tensor("ones1", [65, HD], BF16, kind="ExternalInput").ap()
    vones = nc.dram_tensor("vones", [P, TT, HPG], BF16,
                           kind="ExternalInput").ap()
    outT = nc.dram_tensor("outT", [C, nq], F32, kind="ExternalOutput").ap()
    aps = (qxT, kvxT, wq, wk, wv, wp, msk, colsel, bcast, ones1, vones, outT)
    with _FixedTileContext(nc) as tc:
        _body(tc, aps, nq)
    return nc


def _plan_compaction(attn_mask, nq_min=768):
    """Per-batch unmasked-token indices and a shared padded capacity."""
    sels = [np.nonzero(np.asarray(attn_mask[b]))[0] for b in range(B)]
    cnts = [len(s) for s in sels]
    cap = max(max(cnts) + 2, nq_min)
    cap = ((cap + 255) // 256) * 256
    return sels, cnts, cap


def make_in_maps(q_x, kv_x, attn_mask, Wq, Wkv, Wp, sels, cnts, nq):
    colsel = np.zeros((P, NT, HPG), np.float32)
    bcast = np.zeros((HPG, NT, P), np.float32)
    for t in range(NT):
        for pp in range(P):
            colsel[pp, t, 2 * t + pp // HD] = 1.0 / HD
            bcast[2 * t + pp // HD, t, pp] = 1.0
    ones1 = np.zeros((65, HD), np.float32)
    ones1[[0, 32, 64], :] = 1.0
    bf = lambda a: np.ascontiguousarray(a).astype(NPBF16)

    in_maps = []
    for core in range(NCORES):
        b, g = core // G, core % G
        sl = slice(g * CL, (g + 1) * CL)
        qc = np.zeros((C, nq), np.float32)
        qc[:, 0:cnts[b]] = q_x[b][sels[b]].T
        mv = np.zeros((nq,), np.float32)
        mv[0:cnts[b]] = SCALE
        in_maps.append({
            "qxT": bf(qc),
            "kvxT": bf(kv_x[b].T),
            "wq": bf(Wq[sl].T),
            "wk": bf(Wkv[sl].T),
            "wv": bf(Wkv[C + g * CL:C + (g + 1) * CL].T),
            "wp": bf(Wp[:, sl].T),
            "msk": np.broadcast_to(mv, (HPG, nq)).copy(),
            "colsel": bf(colsel),
            "bcast": bf(bcast),
            "ones1": bf(ones1),
            "vones": bf(np.ones((P, TT, HPG), np.float32)),
        })
    return in_maps


_NC_CACHE = {}


def get_nc(nq):
    if nq not in _NC_CACHE:
        _NC_CACHE[nq] = build_bass(nq)
    return _NC_CACHE[nq]


def kernel(q_x, kv_x, attn_mask, Wq, Wkv, qn_w, qn_b, kn_w, kn_b, Wp, bp,
           _profile=None):
    q_x = np.asarray(q_x, np.float32)
    kv_x = np.asarray(kv_x, np.float32)
    attn_mask = np.asarray(attn_mask)
    Wq = np.asarray(Wq, np.float32)
    Wkv = np.asarray(Wkv, np.float32)
    Wp = np.asarray(Wp, np.float32)
    bp = np.asarray(bp, np.float32)
    if not (np.all(np.asarray(qn_w) == 1) and np.all(np.asarray(qn_b) == 0)
            and np.all(np.asarray(kn_w) == 1) and np.all(np.asarray(kn_b) == 0)):
        raise NotImplementedError("kernel specialized to identity q/k norms")

    sels, cnts, nq = _plan_compaction(attn_mask)
    nc = get_nc(nq)
    in_maps = make_in_maps(q_x, kv_x, attn_mask, Wq, Wkv, Wp, sels, cnts, nq)
    res = bass_utils.run_bass_kernel_spmd(
        nc, in_maps, core_ids=list(range(NCORES)))
    if _profile is not None:
        _profile.append(res)
    out = np.empty((B, N, C), np.float32)
    for b in range(B):
        acc = res.results[G * b]["outT"] + res.results[G * b + 1]["outT"]
        out[b] = acc[:, cnts[b]][None, :] + bp   # uniform row for masked
        out[b, sels[b]] = acc[:, 0:cnts[b]].T + bp
    return out
